# revision 30
# baseline (speedup 1.0000x reference)
"""Adaptive weighted knowledge-distillation loss on 8 TRN2 NeuronCores.

Transposed-layout bf16 design (v4). Data parallel over the batch: each core
owns 512 rows. The host pre-transposes each core's logit shard to
class-major [C, RT] bf16, padding the 512 rows to RT=517 with one all-ones
fake row BEFORE each row-block (blocks of 102,102,102,102,104 real rows).
In this layout a batch row is an SBUF *column*, so the per-row sums that
dominate this loss become partition-contractions on the PE array via
diagonal matmuls, while ACT runs only 2 exp passes (vs 4 row-major) and
DVE runs 3 bf16 2x squarings:

  per row r (T=4):  E1=sum e^t   F1=sum t e^t    E4=sum e^{t/4}
                    A4=sum t e^{t/4}  B4=sum s e^{t/4}
                    H1=sum e^s   H4=sum e^{s/4}  picked=s[target]

  ACT: w4t=exp(t/4), w4s=exp(s/4)          (2 passes)
  DVE: w2t=w4t^2, w1t=w2t^2, w2s=w4s^2     (bf16 tensor_tensor 2x)
  PE per (chunk, block), 15 matmuls/chunk, all tile_size (128,128):
    mm1: lhsT=[1|t], rhs=[w4t|w1t|w4s] (3-slab strided AP, one SBUF
         mega-tile) -> A4/F1 on subdiagonals, E4/E1/H4 on psum row 0.
         Merging the three lhsT=[1|t] matmuls into one amortizes the
         LDWEIGHTS (the load track measured as long as the stream track).
    mm2: lhsT=[1|s], rhs=w4t -> B4 subdiag
    mm3: lhsT=w2s,  rhs=w2s -> H1 subdiag (H1 = sum (e^{s/2})^2)

The ones rows make lhsT col 0 all-ones, so E-sums land on psum partition 0
(compute engines can only address partition starts 0/32/64/96), and real
row j of a block sits at psum partition j+1 — the extraction masks encode
the subdiagonal. PSUM: 5 per-block banks for mm1 + 1 each for B4/H1 + 1
scratch = exactly 8. Host-validated numerics: bf16 end-to-end rel err
8e-5 vs the f32 reference (gate 2e-2).

History (HW measured): row-major baseline (4 ACT exps + 3 DVE STT
passes) 494us, ACT-bound at 94%. v2 gpsimd-H4 1292us
(partition_all_reduce runs 5.7x its cost model; gpsimd now only does the
gather). v3 per-quantity matmuls 416us (PE load+stream tracks ~380us
each). v4 merged-mm1 413us. v5 graded first groups [small->G] 368us
(kills the 30us DMA->exp->square chain before the first matmul; PE
stream 310us busy, ~83%). v6 adds 3-deep input prefetch + constants
DMA'd after the main loop is queued.
"""

import numpy as np
import ml_dtypes

import concourse.bacc as bacc
import concourse.bass as bass
import concourse.tile as tile
from concourse import mybir
from concourse.bass_utils import run_bass_kernel_spmd

B, C = 4096, 32000
NCORES = 8
R = B // NCORES      # 512 real rows per core
P = 128              # SBUF partitions / classes per chunk
NCH = C // P         # 250 class chunks
G = 10               # chunks fused per ACT/DVE instruction group
NG = NCH // G        # 25 groups
GRT = None           # set below

# row blocks: 4x102 + 104 real rows, each PRECEDED by one all-ones fake row
W = [102, 102, 102, 102, 104]          # real rows per block
RS = [0, 102, 204, 306, 408]           # real-row start of each block
PS = [0, 103, 206, 309, 412]           # padded-col start of each block (ones)
FO = [0, 102, 204, 306, 408]           # packed free offset of each block
RT = 517                               # padded rows (512 real + 5 ones)
NB = 5
PMAX = 105                             # max psum partition extent (104+1)
GRT = G * RT

T = 4.0
LN_C = float(np.log(np.float32(C), dtype=np.float32))

FP32 = mybir.dt.float32
BF16 = mybir.dt.bfloat16
I32 = mybir.dt.int32
ALU = mybir.AluOpType
ACTF = mybir.ActivationFunctionType
AX = mybir.AxisListType


def _build_body(tc, tT, sT, goff_d, vmask_d, dmask_d, dmfa_d, out_dram):
    nc = tc.nc

    with (
        tc.tile_pool(name="tin", bufs=2) as tin_pool,
        tc.tile_pool(name="sin", bufs=2) as sin_pool,
        tc.tile_pool(name="wrk", bufs=2) as wrk_pool,
        tc.tile_pool(name="cst", bufs=1) as cst_pool,
        tc.tile_pool(name="fin", bufs=1) as fin_pool,
        tc.tile_pool(name="ps", bufs=1, space="PSUM") as psum_pool,
        tc.tile_pool(name="pst", bufs=1, space="PSUM") as pst_pool,
    ):
        # --- persistent psum accumulators: 5 (per-block FAH) + B + H1 ---
        psFAH = [
            psum_pool.tile([PMAX, 512], FP32, name=f"psFAH{b}", tag=f"psFAH{b}")
            for b in range(NB)
        ]
        psB = psum_pool.tile([PMAX, 512], FP32, tag="psB")
        psH = psum_pool.tile([PMAX, 512], FP32, tag="psH")

        # --- main streaming loop ---
        # graded group sizes: small first groups so PE starts ~25us sooner
        # (the DMA -> exp -> square -> square chain delays the first matmul)
        groups = [3, 7] + [G] * ((NCH - 10) // G)
        assert sum(groups) == NCH
        ci0 = 0
        for gsz in groups:
            grt = gsz * RT
            cls0 = ci0 * P
            tg = tin_pool.tile([P, grt], BF16, name="tg", tag="tg")
            sg = sin_pool.tile([P, grt], BF16, name="sg", tag="sg")
            nc.sync.dma_start(
                out=tg[:].rearrange("p (j r) -> p j r", j=gsz),
                in_=tT[cls0 : cls0 + gsz * P, :].rearrange("(j p) r -> p j r", p=P),
            )
            nc.sync.dma_start(
                out=sg[:].rearrange("p (j r) -> p j r", j=gsz),
                in_=sT[cls0 : cls0 + gsz * P, :].rearrange("(j p) r -> p j r", p=P),
            )
            # mega-tile: [w4t | w1t | w4s] slabs of grt cols each, so mm1's
            # rhs is one strided AP across the three
            # 3 bufs: the exp->square->square chain (~15us) spans more than one
            # PE group window (~12.5us); double-buffering stalls PE
            wme = wrk_pool.tile([P, 3 * grt], BF16, name="wme", tag="wme", bufs=3)
            w4t = wme[:, 0:grt]
            w1t = wme[:, grt : 2 * grt]
            w4s = wme[:, 2 * grt : 3 * grt]
            w2t = wrk_pool.tile([P, grt], BF16, name="w2t", tag="w2t")
            w2s = wrk_pool.tile([P, grt], BF16, name="w2s", tag="w2s")
            nc.scalar.activation(out=w4t, in_=tg[:], func=ACTF.Exp, scale=1.0 / T)
            nc.scalar.activation(out=w4s, in_=sg[:], func=ACTF.Exp, scale=1.0 / T)
            nc.vector.tensor_tensor(out=w2t[:], in0=w4t, in1=w4t, op=ALU.mult)
            nc.vector.tensor_tensor(out=w1t, in0=w2t[:], in1=w2t[:], op=ALU.mult)
            nc.vector.tensor_tensor(out=w2s[:], in0=w4s, in1=w4s, op=ALU.mult)
            wme3 = wme[:].rearrange("p (s x) -> p s x", s=3)

            for j in range(gsz):
                ci = ci0 + j
                start = ci == 0
                stop = ci == NCH - 1
                co = j * RT
                for b in range(NB):
                    lo = co + PS[b]
                    w = W[b]
                    fs = slice(FO[b], FO[b] + w)
                    rr = slice(lo + 1, lo + 1 + w)    # real cols only
                    nc.tensor.matmul(
                        out=psFAH[b][0 : w + 1, 0 : 3 * w],
                        lhsT=tg[:, lo : lo + w + 1],  # ones col + real rows
                        rhs=wme3[:, :, rr],
                        start=start, stop=stop,
                    )
                    nc.tensor.matmul(
                        out=psB[0 : w + 1, fs],
                        lhsT=sg[:, lo : lo + w + 1],
                        rhs=w4t[:, rr],
                        start=start, stop=stop,
                    )
                    nc.tensor.matmul(
                        out=psH[0 : w + 1, fs],
                        lhsT=w2s[:, lo : lo + w + 1],
                        rhs=w2s[:, rr],
                        start=start, stop=stop,
                    )
            ci0 += gsz

        # --- constants ---
        goff = cst_pool.tile([PMAX, NB], I32, tag="goff")
        nc.sync.dma_start(out=goff[:], in_=goff_d[:])
        vmask = cst_pool.tile([PMAX, NB], FP32, tag="vmask")
        nc.sync.dma_start(out=vmask[:], in_=vmask_d[:])
        dmask = cst_pool.tile([PMAX, 512], FP32, tag="dmask")
        nc.sync.dma_start(out=dmask[:], in_=dmask_d[:])
        dmfa = cst_pool.tile([PMAX, NB * 512], FP32, tag="dmfa")
        nc.sync.dma_start(out=dmfa[:], in_=dmfa_d[:])
        one1 = cst_pool.tile([1, 1], FP32, tag="one1")
        nc.vector.memset(one1[:], 1.0)

        # --- target gather: picked[i, b] = sT[tgt*RT + padded_pos] ---
        picked_bf = cst_pool.tile([PMAX, NB], BF16, tag="picked_bf")
        s_flat = sT[:].rearrange("c r -> (c r)")[:, None]
        for b in range(NB):
            nc.gpsimd.indirect_dma_start(
                out=picked_bf[:, b : b + 1],
                out_offset=None,
                in_=s_flat,
                in_offset=bass.IndirectOffsetOnAxis(ap=goff[:, b : b + 1], axis=0),
            )

        # --- extraction: per-row quantities as [PMAX, NB] f32 tiles ---
        def ftile(name, dt=FP32):
            return fin_pool.tile([PMAX, NB], dt, name=f"f_{name}", tag=f"f_{name}")

        # A4/F1 subdiagonals from the per-block FAH banks
        qF, qA = ftile("F1"), ftile("A4")
        for b in range(NB):
            w = W[b]
            m = fin_pool.tile([PMAX, 512], FP32, name="dm", tag="dm")
            nc.vector.tensor_tensor(
                out=m[:, 0 : 2 * w],
                in0=psFAH[b][:, 0 : 2 * w],
                in1=dmfa[:, b * 512 : b * 512 + 2 * w],
                op=ALU.mult,
            )
            nc.vector.reduce_sum(out=qA[:, b : b + 1], in_=m[:, 0:w], axis=AX.X)
            nc.vector.reduce_sum(out=qF[:, b : b + 1], in_=m[:, w : 2 * w], axis=AX.X)

        # B4/H1 subdiagonals
        qB, qH1 = ftile("B4"), ftile("H1")
        for q, ps in ((qB, psB), (qH1, psH)):
            m = fin_pool.tile([PMAX, 512], FP32, name="dm", tag="dm")
            nc.vector.tensor_tensor(out=m[:], in0=ps[:], in1=dmask[:], op=ALU.mult)
            for b in range(NB):
                nc.vector.reduce_sum(
                    out=q[:, b : b + 1],
                    in_=m[:, FO[b] : FO[b] + W[b]],
                    axis=AX.X,
                )

        # E4/E1/H4 rows (psum partition 0 of each FAH bank) -> columns via
        # 1-partition matmul transpose, shifted one col so row j lands at
        # partition j+1 (matching the subdiagonal layout)
        qE1, qE4, qH4 = ftile("E1"), ftile("E4"), ftile("H4")
        rows = {}
        for nm in ("e4", "e1", "h4"):
            rows[nm] = fin_pool.tile([1, 520], FP32, name=f"r_{nm}", tag=f"r_{nm}")
            nc.vector.memset(rows[nm][:], 1.0)
        for b in range(NB):
            w = W[b]
            nc.scalar.copy(
                out=rows["e4"][0:1, 1 + FO[b] : 1 + FO[b] + w],
                in_=psFAH[b][0:1, 0:w],
            )
            nc.scalar.copy(
                out=rows["e1"][0:1, 1 + FO[b] : 1 + FO[b] + w],
                in_=psFAH[b][0:1, w : 2 * w],
            )
            nc.scalar.copy(
                out=rows["h4"][0:1, 1 + FO[b] : 1 + FO[b] + w],
                in_=psFAH[b][0:1, 2 * w : 3 * w],
            )
        for q, nm in ((qE1, "e1"), (qE4, "e4"), (qH4, "h4")):
            nc.vector.memset(q[:], 1.0)
            for b in range(NB):
                w = W[b]
                pt = pst_pool.tile([PMAX, 1], FP32, name="pt", tag="pt")
                nc.tensor.matmul(
                    out=pt[0 : w + 1, 0:1],
                    lhsT=rows[nm][0:1, FO[b] : FO[b] + w + 1],
                    rhs=one1[:],
                    start=True, stop=True,
                )
                nc.vector.tensor_copy(
                    out=q[0 : w + 1, b : b + 1], in_=pt[0 : w + 1, 0:1]
                )

        picked = ftile("picked")
        nc.vector.tensor_copy(out=picked[:], in_=picked_bf[:])

        # qH1's invalid slots (ones row / pad partitions) extract as 0 via
        # the diag mask; bump them to 1 so Ln stays finite (masked later)
        ivm = ftile("ivm")
        nc.vector.tensor_scalar(
            out=ivm[:], in0=vmask[:], scalar1=-1.0, scalar2=1.0,
            op0=ALU.mult, op1=ALU.add,
        )
        nc.vector.tensor_tensor(out=qH1[:], in0=qH1[:], in1=ivm[:], op=ALU.add)

        # --- per-row finalize on [PMAX, NB] ---
        rE1, rE4 = ftile("rE1"), ftile("rE4")
        nc.vector.reciprocal(out=rE1[:], in_=qE1[:])
        nc.vector.reciprocal(out=rE4[:], in_=qE4[:])
        logs = {}
        for nm, q in (("E1", qE1), ("E4", qE4), ("H1", qH1), ("H4", qH4)):
            logs[nm] = ftile(f"log{nm}")
            nc.scalar.activation(out=logs[nm][:], in_=q[:], func=ACTF.Ln)

        ent = ftile("ent")
        nc.vector.tensor_tensor(out=ent[:], in0=qF[:], in1=rE1[:], op=ALU.mult)
        nc.vector.tensor_tensor(out=ent[:], in0=logs["E1"][:], in1=ent[:], op=ALU.subtract)
        alpha = ftile("alpha")
        nc.vector.tensor_scalar(
            out=alpha[:], in0=ent[:],
            scalar1=-1.0 / LN_C, scalar2=1.0, op0=ALU.mult, op1=ALU.add,
        )
        nc.vector.tensor_scalar_max(out=alpha[:], in0=alpha[:], scalar1=0.0)
        nc.vector.tensor_scalar_min(out=alpha[:], in0=alpha[:], scalar1=1.0)

        ce = ftile("ce")
        nc.vector.tensor_tensor(out=ce[:], in0=logs["H1"][:], in1=picked[:], op=ALU.subtract)

        kl = ftile("kl")
        nc.vector.tensor_tensor(out=kl[:], in0=qA[:], in1=qB[:], op=ALU.subtract)
        nc.vector.tensor_tensor(out=kl[:], in0=kl[:], in1=rE4[:], op=ALU.mult)
        nc.vector.tensor_scalar_mul(out=kl[:], in0=kl[:], scalar1=1.0 / T)
        nc.vector.tensor_tensor(out=kl[:], in0=kl[:], in1=logs["E4"][:], op=ALU.subtract)
        nc.vector.tensor_tensor(out=kl[:], in0=kl[:], in1=logs["H4"][:], op=ALU.add)

        # loss = ce + alpha*(T^2*kl - ce), then mask out fake/pad rows
        loss = ftile("loss")
        nc.vector.tensor_scalar_mul(out=loss[:], in0=kl[:], scalar1=T * T)
        nc.vector.tensor_tensor(out=loss[:], in0=loss[:], in1=ce[:], op=ALU.subtract)
        nc.vector.tensor_tensor(out=loss[:], in0=loss[:], in1=alpha[:], op=ALU.mult)
        nc.vector.tensor_tensor(out=loss[:], in0=loss[:], in1=ce[:], op=ALU.add)
        nc.vector.tensor_tensor(out=loss[:], in0=loss[:], in1=vmask[:], op=ALU.mult)

        rowsum = fin_pool.tile([PMAX, 1], FP32, tag="f_rowsum")
        nc.vector.reduce_sum(out=rowsum[:], in_=loss[:], axis=AX.X)
        onesB = fin_pool.tile([PMAX, 1], FP32, tag="f_onesB")
        nc.vector.memset(onesB[:], 1.0 / B)
        part_ps = pst_pool.tile([PMAX, 1], FP32, name="pt", tag="pt")
        nc.tensor.matmul(
            out=part_ps[0:1, 0:1], lhsT=rowsum[:], rhs=onesB[:], start=True, stop=True
        )
        part_sb = fin_pool.tile([1, 1], FP32, tag="f_part")
        nc.vector.tensor_copy(out=part_sb[:], in_=part_ps[0:1, 0:1])
        nc.sync.dma_start(out=out_dram[:], in_=part_sb[:])


_CACHED_NC = None


def _build():
    global _CACHED_NC
    if _CACHED_NC is not None:
        return _CACHED_NC
    nc = bacc.Bacc(
        "TRN2", target_bir_lowering=False, debug=False, num_devices=NCORES
    )
    tT = nc.dram_tensor("teacher_t", [C, RT], BF16, kind="ExternalInput")
    sT = nc.dram_tensor("student_t", [C, RT], BF16, kind="ExternalInput")
    goff_d = nc.dram_tensor("goff", [PMAX, NB], I32, kind="ExternalInput")
    vmask_d = nc.dram_tensor("vmask", [PMAX, NB], FP32, kind="ExternalInput")
    dmask_d = nc.dram_tensor("dmask", [PMAX, 512], FP32, kind="ExternalInput")
    dmfa_d = nc.dram_tensor("dmask_fa", [PMAX, NB * 512], FP32, kind="ExternalInput")
    out_dram = nc.dram_tensor("out", [1, 1], FP32, kind="ExternalOutput")
    with nc.allow_low_precision(reason="bf16 pipeline host-validated: 8e-5 rel err"):
        with tile.TileContext(nc) as tc:
            _build_body(tc, tT, sT, goff_d, vmask_d, dmask_d, dmfa_d, out_dram[:])
    nc.compile()
    _CACHED_NC = nc
    return nc


def _host_prep(outputs, teacher_outputs, targets):
    """Pad rows to RT with ones rows, transpose to class-major, cast bf16."""
    bf16 = ml_dtypes.bfloat16
    tgt = np.asarray(targets).astype(np.int64).reshape(B)

    # padded row layout per core: ones row first, then the block's real rows
    pad_pos = np.zeros(R, dtype=np.int64)     # real row -> padded col
    for b in range(NB):
        pad_pos[RS[b] : RS[b] + W[b]] = PS[b] + 1 + np.arange(W[b])

    # masks (identical across cores); real row j of block b sits at
    # psum/fin partition j+1 (partition 0 is the ones row)
    vmask = np.zeros((PMAX, NB), dtype=np.float32)
    dmask = np.zeros((PMAX, 512), dtype=np.float32)
    dmask_fa = np.zeros((PMAX, NB * 512), dtype=np.float32)
    for b in range(NB):
        w = W[b]
        vmask[1 : w + 1, b] = 1.0
        dmask[1 + np.arange(w), FO[b] + np.arange(w)] = 1.0
        dmask_fa[1 + np.arange(w), b * 512 + np.arange(w)] = 1.0
        dmask_fa[1 + np.arange(w), b * 512 + w + np.arange(w)] = 1.0

    t16 = np.asarray(teacher_outputs, dtype=np.float32).astype(bf16)
    s16 = np.asarray(outputs, dtype=np.float32).astype(bf16)

    in_maps = []
    for i in range(NCORES):
        sl = slice(i * R, (i + 1) * R)
        tpad = np.ones((RT, C), dtype=bf16)
        spad = np.ones((RT, C), dtype=bf16)
        tpad[pad_pos] = t16[sl]
        spad[pad_pos] = s16[sl]
        tTc = np.ascontiguousarray(tpad.T)
        sTc = np.ascontiguousarray(spad.T)

        goff = np.zeros((PMAX, NB), dtype=np.int32)
        tgt_c = tgt[sl]
        for b in range(NB):
            rows = RS[b] + np.arange(W[b])
            goff[1 : W[b] + 1, b] = tgt_c[rows] * RT + PS[b] + 1 + np.arange(W[b])

        in_maps.append(
            {
                "teacher_t": tTc,
                "student_t": sTc,
                "goff": goff,
                "vmask": vmask,
                "dmask": dmask,
                "dmask_fa": dmask_fa,
            }
        )
    return in_maps


def kernel(outputs, teacher_outputs, targets, _results_out=None):
    assert np.asarray(outputs).shape == (B, C)
    in_maps = _host_prep(outputs, teacher_outputs, targets)
    nc = _build()
    res = run_bass_kernel_spmd(nc, in_maps, core_ids=list(range(NCORES)))
    if _results_out is not None:
        _results_out.append(res)
    # gather/unshard: each core returns its (local loss sum)/B partial
    return np.float32(sum(np.float32(r["out"].reshape(())) for r in res.results))


# revision 31
# speedup vs baseline: 1.2273x; 1.2273x over previous
"""Adaptive weighted knowledge-distillation loss on 8 TRN2 NeuronCores.

Transposed-layout bf16 design (v4). Data parallel over the batch: each core
owns 512 rows. The host pre-transposes each core's logit shard to
class-major [C, RT] bf16, padding the 512 rows to RT=517 with one all-ones
fake row BEFORE each row-block (blocks of 102,102,102,102,104 real rows).
In this layout a batch row is an SBUF *column*, so the per-row sums that
dominate this loss become partition-contractions on the PE array via
diagonal matmuls, while ACT runs only 2 exp passes (vs 4 row-major) and
DVE runs 3 bf16 2x squarings:

  per row r (T=4):  E1=sum e^t   F1=sum t e^t    E4=sum e^{t/4}
                    A4=sum t e^{t/4}  B4=sum s e^{t/4}
                    H1=sum e^s   H4=sum e^{s/4}  picked=s[target]

  ACT: w4t=exp(t/4), w4s=exp(s/4)          (2 passes)
  DVE: w2t=w4t^2, w1t=w2t^2, w2s=w4s^2     (bf16 tensor_tensor 2x)
  PE per (chunk, block), 15 matmuls/chunk, all tile_size (128,128):
    mm1: lhsT=[1|t], rhs=[w4t|w1t|w4s] (3-slab strided AP, one SBUF
         mega-tile) -> A4/F1 on subdiagonals, E4/E1/H4 on psum row 0.
         Merging the three lhsT=[1|t] matmuls into one amortizes the
         LDWEIGHTS (the load track measured as long as the stream track).
    mm2: lhsT=[1|s], rhs=w4t -> B4 subdiag
    mm3: lhsT=w2s,  rhs=w2s -> H1 subdiag (H1 = sum (e^{s/2})^2)

The ones rows make lhsT col 0 all-ones, so E-sums land on psum partition 0
(compute engines can only address partition starts 0/32/64/96), and real
row j of a block sits at psum partition j+1 — the extraction masks encode
the subdiagonal. PSUM: 5 per-block banks for mm1 + 1 each for B4/H1 + 1
scratch = exactly 8. Host-validated numerics: bf16 end-to-end rel err
8e-5 vs the f32 reference (gate 2e-2).

History (HW measured): row-major baseline (4 ACT exps + 3 DVE STT
passes) 494us, ACT-bound at 94%. v2 gpsimd-H4 1292us
(partition_all_reduce runs 5.7x its cost model; gpsimd now only does the
gather). v3 per-quantity matmuls 416us (PE load+stream tracks ~380us
each). v4 merged-mm1 413us. v5 graded first groups [small->G] 368us
(kills the 30us DMA->exp->square chain before the first matmul; PE
stream 310us busy, ~83%). v6 adds 3-deep input prefetch + constants
DMA'd after the main loop is queued.
"""

import numpy as np
import ml_dtypes

import concourse.bacc as bacc
import concourse.bass as bass
import concourse.tile as tile
from concourse import mybir
from concourse.bass_utils import run_bass_kernel_spmd

B, C = 4096, 32000
NCORES = 8
R = B // NCORES      # 512 real rows per core
P = 128              # SBUF partitions / classes per chunk
NCH = C // P         # 250 class chunks
G = 10               # chunks fused per ACT/DVE instruction group
NG = NCH // G        # 25 groups
GRT = None           # set below

# row blocks: 4x102 + 104 real rows, each PRECEDED by one all-ones fake row
W = [102, 102, 102, 102, 104]          # real rows per block
RS = [0, 102, 204, 306, 408]           # real-row start of each block
PS = [0, 103, 206, 309, 412]           # padded-col start of each block (ones)
FO = [0, 102, 204, 306, 408]           # packed free offset of each block
RT = 517                               # padded rows (512 real + 5 ones)
NB = 5
PMAX = 105                             # max psum partition extent (104+1)
GRT = G * RT

T = 4.0
LN_C = float(np.log(np.float32(C), dtype=np.float32))

FP32 = mybir.dt.float32
BF16 = mybir.dt.bfloat16
I32 = mybir.dt.int32
ALU = mybir.AluOpType
ACTF = mybir.ActivationFunctionType
AX = mybir.AxisListType


def _build_body(tc, tT, sT, goff_d, vmask_d, dmask_d, dmfa_d, out_dram):
    nc = tc.nc

    with (
        tc.tile_pool(name="tin", bufs=3) as tin_pool,
        tc.tile_pool(name="sin", bufs=3) as sin_pool,
        tc.tile_pool(name="wrk", bufs=2) as wrk_pool,
        tc.tile_pool(name="cst", bufs=1) as cst_pool,
        tc.tile_pool(name="fin", bufs=1) as fin_pool,
        tc.tile_pool(name="ps", bufs=1, space="PSUM") as psum_pool,
        tc.tile_pool(name="pst", bufs=1, space="PSUM") as pst_pool,
    ):
        # --- persistent psum accumulators: 5 (per-block FAH) + B + H1 ---
        psFAH = [
            psum_pool.tile([PMAX, 512], FP32, name=f"psFAH{b}", tag=f"psFAH{b}")
            for b in range(NB)
        ]
        psB = psum_pool.tile([PMAX, 512], FP32, tag="psB")
        psH = psum_pool.tile([PMAX, 512], FP32, tag="psH")

        # --- main streaming loop ---
        # graded group sizes: small first groups so PE starts ~25us sooner
        # (the DMA -> exp -> square -> square chain delays the first matmul)
        groups = [3, 7] + [G] * ((NCH - 10) // G)
        assert sum(groups) == NCH
        ci0 = 0
        for gsz in groups:
            grt = gsz * RT
            cls0 = ci0 * P
            tg = tin_pool.tile([P, grt], BF16, name="tg", tag="tg")
            sg = sin_pool.tile([P, grt], BF16, name="sg", tag="sg")
            nc.sync.dma_start(
                out=tg[:].rearrange("p (j r) -> p j r", j=gsz),
                in_=tT[cls0 : cls0 + gsz * P, :].rearrange("(j p) r -> p j r", p=P),
            )
            nc.sync.dma_start(
                out=sg[:].rearrange("p (j r) -> p j r", j=gsz),
                in_=sT[cls0 : cls0 + gsz * P, :].rearrange("(j p) r -> p j r", p=P),
            )
            # mega-tile: [w4t | w1t | w4s] slabs of grt cols each, so mm1's
            # rhs is one strided AP across the three
            wme = wrk_pool.tile([P, 3 * grt], BF16, name="wme", tag="wme")
            w4t = wme[:, 0:grt]
            w1t = wme[:, grt : 2 * grt]
            w4s = wme[:, 2 * grt : 3 * grt]
            w2t = wrk_pool.tile([P, grt], BF16, name="w2t", tag="w2t")
            w2s = wrk_pool.tile([P, grt], BF16, name="w2s", tag="w2s")
            nc.scalar.activation(out=w4t, in_=tg[:], func=ACTF.Exp, scale=1.0 / T)
            nc.scalar.activation(out=w4s, in_=sg[:], func=ACTF.Exp, scale=1.0 / T)
            nc.vector.tensor_tensor(out=w2t[:], in0=w4t, in1=w4t, op=ALU.mult)
            nc.vector.tensor_tensor(out=w1t, in0=w2t[:], in1=w2t[:], op=ALU.mult)
            nc.vector.tensor_tensor(out=w2s[:], in0=w4s, in1=w4s, op=ALU.mult)
            wme3 = wme[:].rearrange("p (s x) -> p s x", s=3)

            for j in range(gsz):
                ci = ci0 + j
                start = ci == 0
                stop = ci == NCH - 1
                co = j * RT
                for b in range(NB):
                    lo = co + PS[b]
                    w = W[b]
                    fs = slice(FO[b], FO[b] + w)
                    rr = slice(lo + 1, lo + 1 + w)    # real cols only
                    nc.tensor.matmul(
                        out=psFAH[b][0 : w + 1, 0 : 3 * w],
                        lhsT=tg[:, lo : lo + w + 1],  # ones col + real rows
                        rhs=wme3[:, :, rr],
                        start=start, stop=stop,
                    )
                    nc.tensor.matmul(
                        out=psB[0 : w + 1, fs],
                        lhsT=sg[:, lo : lo + w + 1],
                        rhs=w4t[:, rr],
                        start=start, stop=stop,
                    )
                    nc.tensor.matmul(
                        out=psH[0 : w + 1, fs],
                        lhsT=w2s[:, lo : lo + w + 1],
                        rhs=w2s[:, rr],
                        start=start, stop=stop,
                    )
            ci0 += gsz

        # --- constants ---
        goff = cst_pool.tile([PMAX, NB], I32, tag="goff")
        nc.sync.dma_start(out=goff[:], in_=goff_d[:])
        vmask = cst_pool.tile([PMAX, NB], FP32, tag="vmask")
        nc.sync.dma_start(out=vmask[:], in_=vmask_d[:])
        dmask = cst_pool.tile([PMAX, 512], FP32, tag="dmask")
        nc.sync.dma_start(out=dmask[:], in_=dmask_d[:])
        dmfa = cst_pool.tile([PMAX, NB * 512], FP32, tag="dmfa")
        nc.sync.dma_start(out=dmfa[:], in_=dmfa_d[:])
        one1 = cst_pool.tile([1, 1], FP32, tag="one1")
        nc.vector.memset(one1[:], 1.0)

        # --- target gather: picked[i, b] = sT[tgt*RT + padded_pos] ---
        picked_bf = cst_pool.tile([PMAX, NB], BF16, tag="picked_bf")
        s_flat = sT[:].rearrange("c r -> (c r)")[:, None]
        for b in range(NB):
            nc.gpsimd.indirect_dma_start(
                out=picked_bf[:, b : b + 1],
                out_offset=None,
                in_=s_flat,
                in_offset=bass.IndirectOffsetOnAxis(ap=goff[:, b : b + 1], axis=0),
            )

        # --- extraction: per-row quantities as [PMAX, NB] f32 tiles ---
        def ftile(name, dt=FP32):
            return fin_pool.tile([PMAX, NB], dt, name=f"f_{name}", tag=f"f_{name}")

        # A4/F1 subdiagonals from the per-block FAH banks
        qF, qA = ftile("F1"), ftile("A4")
        for b in range(NB):
            w = W[b]
            m = fin_pool.tile([PMAX, 512], FP32, name="dm", tag="dm")
            nc.vector.tensor_tensor(
                out=m[:, 0 : 2 * w],
                in0=psFAH[b][:, 0 : 2 * w],
                in1=dmfa[:, b * 512 : b * 512 + 2 * w],
                op=ALU.mult,
            )
            nc.vector.reduce_sum(out=qA[:, b : b + 1], in_=m[:, 0:w], axis=AX.X)
            nc.vector.reduce_sum(out=qF[:, b : b + 1], in_=m[:, w : 2 * w], axis=AX.X)

        # B4/H1 subdiagonals
        qB, qH1 = ftile("B4"), ftile("H1")
        for q, ps in ((qB, psB), (qH1, psH)):
            m = fin_pool.tile([PMAX, 512], FP32, name="dm", tag="dm")
            nc.vector.tensor_tensor(out=m[:], in0=ps[:], in1=dmask[:], op=ALU.mult)
            for b in range(NB):
                nc.vector.reduce_sum(
                    out=q[:, b : b + 1],
                    in_=m[:, FO[b] : FO[b] + W[b]],
                    axis=AX.X,
                )

        # E4/E1/H4 rows (psum partition 0 of each FAH bank) -> columns via
        # 1-partition matmul transpose, shifted one col so row j lands at
        # partition j+1 (matching the subdiagonal layout)
        qE1, qE4, qH4 = ftile("E1"), ftile("E4"), ftile("H4")
        rows = {}
        for nm in ("e4", "e1", "h4"):
            rows[nm] = fin_pool.tile([1, 520], FP32, name=f"r_{nm}", tag=f"r_{nm}")
            nc.vector.memset(rows[nm][:], 1.0)
        for b in range(NB):
            w = W[b]
            nc.scalar.copy(
                out=rows["e4"][0:1, 1 + FO[b] : 1 + FO[b] + w],
                in_=psFAH[b][0:1, 0:w],
            )
            nc.scalar.copy(
                out=rows["e1"][0:1, 1 + FO[b] : 1 + FO[b] + w],
                in_=psFAH[b][0:1, w : 2 * w],
            )
            nc.scalar.copy(
                out=rows["h4"][0:1, 1 + FO[b] : 1 + FO[b] + w],
                in_=psFAH[b][0:1, 2 * w : 3 * w],
            )
        for q, nm in ((qE1, "e1"), (qE4, "e4"), (qH4, "h4")):
            nc.vector.memset(q[:], 1.0)
            for b in range(NB):
                w = W[b]
                pt = pst_pool.tile([PMAX, 1], FP32, name="pt", tag="pt")
                nc.tensor.matmul(
                    out=pt[0 : w + 1, 0:1],
                    lhsT=rows[nm][0:1, FO[b] : FO[b] + w + 1],
                    rhs=one1[:],
                    start=True, stop=True,
                )
                nc.vector.tensor_copy(
                    out=q[0 : w + 1, b : b + 1], in_=pt[0 : w + 1, 0:1]
                )

        picked = ftile("picked")
        nc.vector.tensor_copy(out=picked[:], in_=picked_bf[:])

        # qH1's invalid slots (ones row / pad partitions) extract as 0 via
        # the diag mask; bump them to 1 so Ln stays finite (masked later)
        ivm = ftile("ivm")
        nc.vector.tensor_scalar(
            out=ivm[:], in0=vmask[:], scalar1=-1.0, scalar2=1.0,
            op0=ALU.mult, op1=ALU.add,
        )
        nc.vector.tensor_tensor(out=qH1[:], in0=qH1[:], in1=ivm[:], op=ALU.add)

        # --- per-row finalize on [PMAX, NB] ---
        rE1, rE4 = ftile("rE1"), ftile("rE4")
        nc.vector.reciprocal(out=rE1[:], in_=qE1[:])
        nc.vector.reciprocal(out=rE4[:], in_=qE4[:])
        logs = {}
        for nm, q in (("E1", qE1), ("E4", qE4), ("H1", qH1), ("H4", qH4)):
            logs[nm] = ftile(f"log{nm}")
            nc.scalar.activation(out=logs[nm][:], in_=q[:], func=ACTF.Ln)

        ent = ftile("ent")
        nc.vector.tensor_tensor(out=ent[:], in0=qF[:], in1=rE1[:], op=ALU.mult)
        nc.vector.tensor_tensor(out=ent[:], in0=logs["E1"][:], in1=ent[:], op=ALU.subtract)
        alpha = ftile("alpha")
        nc.vector.tensor_scalar(
            out=alpha[:], in0=ent[:],
            scalar1=-1.0 / LN_C, scalar2=1.0, op0=ALU.mult, op1=ALU.add,
        )
        nc.vector.tensor_scalar_max(out=alpha[:], in0=alpha[:], scalar1=0.0)
        nc.vector.tensor_scalar_min(out=alpha[:], in0=alpha[:], scalar1=1.0)

        ce = ftile("ce")
        nc.vector.tensor_tensor(out=ce[:], in0=logs["H1"][:], in1=picked[:], op=ALU.subtract)

        kl = ftile("kl")
        nc.vector.tensor_tensor(out=kl[:], in0=qA[:], in1=qB[:], op=ALU.subtract)
        nc.vector.tensor_tensor(out=kl[:], in0=kl[:], in1=rE4[:], op=ALU.mult)
        nc.vector.tensor_scalar_mul(out=kl[:], in0=kl[:], scalar1=1.0 / T)
        nc.vector.tensor_tensor(out=kl[:], in0=kl[:], in1=logs["E4"][:], op=ALU.subtract)
        nc.vector.tensor_tensor(out=kl[:], in0=kl[:], in1=logs["H4"][:], op=ALU.add)

        # loss = ce + alpha*(T^2*kl - ce), then mask out fake/pad rows
        loss = ftile("loss")
        nc.vector.tensor_scalar_mul(out=loss[:], in0=kl[:], scalar1=T * T)
        nc.vector.tensor_tensor(out=loss[:], in0=loss[:], in1=ce[:], op=ALU.subtract)
        nc.vector.tensor_tensor(out=loss[:], in0=loss[:], in1=alpha[:], op=ALU.mult)
        nc.vector.tensor_tensor(out=loss[:], in0=loss[:], in1=ce[:], op=ALU.add)
        nc.vector.tensor_tensor(out=loss[:], in0=loss[:], in1=vmask[:], op=ALU.mult)

        rowsum = fin_pool.tile([PMAX, 1], FP32, tag="f_rowsum")
        nc.vector.reduce_sum(out=rowsum[:], in_=loss[:], axis=AX.X)
        onesB = fin_pool.tile([PMAX, 1], FP32, tag="f_onesB")
        nc.vector.memset(onesB[:], 1.0 / B)
        part_ps = pst_pool.tile([PMAX, 1], FP32, name="pt", tag="pt")
        nc.tensor.matmul(
            out=part_ps[0:1, 0:1], lhsT=rowsum[:], rhs=onesB[:], start=True, stop=True
        )
        part_sb = fin_pool.tile([1, 1], FP32, tag="f_part")
        nc.vector.tensor_copy(out=part_sb[:], in_=part_ps[0:1, 0:1])
        nc.sync.dma_start(out=out_dram[:], in_=part_sb[:])


_CACHED_NC = None


def _build():
    global _CACHED_NC
    if _CACHED_NC is not None:
        return _CACHED_NC
    nc = bacc.Bacc(
        "TRN2", target_bir_lowering=False, debug=False, num_devices=NCORES
    )
    tT = nc.dram_tensor("teacher_t", [C, RT], BF16, kind="ExternalInput")
    sT = nc.dram_tensor("student_t", [C, RT], BF16, kind="ExternalInput")
    goff_d = nc.dram_tensor("goff", [PMAX, NB], I32, kind="ExternalInput")
    vmask_d = nc.dram_tensor("vmask", [PMAX, NB], FP32, kind="ExternalInput")
    dmask_d = nc.dram_tensor("dmask", [PMAX, 512], FP32, kind="ExternalInput")
    dmfa_d = nc.dram_tensor("dmask_fa", [PMAX, NB * 512], FP32, kind="ExternalInput")
    out_dram = nc.dram_tensor("out", [1, 1], FP32, kind="ExternalOutput")
    with nc.allow_low_precision(reason="bf16 pipeline host-validated: 8e-5 rel err"):
        with tile.TileContext(nc) as tc:
            _build_body(tc, tT, sT, goff_d, vmask_d, dmask_d, dmfa_d, out_dram[:])
    nc.compile()
    _CACHED_NC = nc
    return nc


def _host_prep(outputs, teacher_outputs, targets):
    """Pad rows to RT with ones rows, transpose to class-major, cast bf16."""
    bf16 = ml_dtypes.bfloat16
    tgt = np.asarray(targets).astype(np.int64).reshape(B)

    # padded row layout per core: ones row first, then the block's real rows
    pad_pos = np.zeros(R, dtype=np.int64)     # real row -> padded col
    for b in range(NB):
        pad_pos[RS[b] : RS[b] + W[b]] = PS[b] + 1 + np.arange(W[b])

    # masks (identical across cores); real row j of block b sits at
    # psum/fin partition j+1 (partition 0 is the ones row)
    vmask = np.zeros((PMAX, NB), dtype=np.float32)
    dmask = np.zeros((PMAX, 512), dtype=np.float32)
    dmask_fa = np.zeros((PMAX, NB * 512), dtype=np.float32)
    for b in range(NB):
        w = W[b]
        vmask[1 : w + 1, b] = 1.0
        dmask[1 + np.arange(w), FO[b] + np.arange(w)] = 1.0
        dmask_fa[1 + np.arange(w), b * 512 + np.arange(w)] = 1.0
        dmask_fa[1 + np.arange(w), b * 512 + w + np.arange(w)] = 1.0

    t16 = np.asarray(teacher_outputs, dtype=np.float32).astype(bf16)
    s16 = np.asarray(outputs, dtype=np.float32).astype(bf16)

    in_maps = []
    for i in range(NCORES):
        sl = slice(i * R, (i + 1) * R)
        tpad = np.ones((RT, C), dtype=bf16)
        spad = np.ones((RT, C), dtype=bf16)
        tpad[pad_pos] = t16[sl]
        spad[pad_pos] = s16[sl]
        tTc = np.ascontiguousarray(tpad.T)
        sTc = np.ascontiguousarray(spad.T)

        goff = np.zeros((PMAX, NB), dtype=np.int32)
        tgt_c = tgt[sl]
        for b in range(NB):
            rows = RS[b] + np.arange(W[b])
            goff[1 : W[b] + 1, b] = tgt_c[rows] * RT + PS[b] + 1 + np.arange(W[b])

        in_maps.append(
            {
                "teacher_t": tTc,
                "student_t": sTc,
                "goff": goff,
                "vmask": vmask,
                "dmask": dmask,
                "dmask_fa": dmask_fa,
            }
        )
    return in_maps


def kernel(outputs, teacher_outputs, targets, _results_out=None):
    assert np.asarray(outputs).shape == (B, C)
    in_maps = _host_prep(outputs, teacher_outputs, targets)
    nc = _build()
    res = run_bass_kernel_spmd(nc, in_maps, core_ids=list(range(NCORES)))
    if _results_out is not None:
        _results_out.append(res)
    # gather/unshard: each core returns its (local loss sum)/B partial
    return np.float32(sum(np.float32(r["out"].reshape(())) for r in res.results))
